# revision 26
# baseline (speedup 1.0000x reference)
"""Trainium2 Bass kernel for nn_CameraEstimator.

Computes, for each batch item b:
    camera[b] = einsum('chw,c->hw', x[b], W)          (C=256 contraction)
    out[b]    = nearest-rotation(camera[b])           (SVD u@vh + det reflection fix)

The SVD-based orthonormalization is replaced by a determinant-scaled Newton
polar iteration plus a closed-form smallest-eigenvalue reflection correction:
    orth = polar(camera)            (Newton: Y <- 0.5*(mu*Y + (mu*det)^-1 * cof(Y)))
    P = orth^T camera = V S V^T;  s3 = smallest eig of P (trig formula + polish)
    proj = adj(P - s3 I)/tr(...) = v3 v3^T
    out = orth - (1 - sign(det)) * orth @ proj

Sharding: batch dim split evenly across 8 NeuronCores (data parallel), W
replicated. All math is done on [128, TPC, 9]-shaped fp32 SBUF planes
(partition = b mod, free = (tile, matrix-entry)).
"""

import os
import numpy as np

import concourse.bacc as bacc
import concourse.bass as bass
import concourse.mybir as mybir
from concourse.bass_types import AP
from concourse.tile import TileContext
from concourse import bass_utils

F32 = mybir.dt.float32
ALU = mybir.AluOpType
ACT = mybir.ActivationFunctionType

B_FULL = 32768
C = 256
E = 9
N_CORES = 8
P = 128
B_LOCAL = B_FULL // N_CORES          # 4096
TPC = B_LOCAL // P                   # 32 matrices per partition

NEWTON_ITERS = 6
SCALED_ITERS = 3
POLISH_ITERS = 2


def v(base: AP, off: int, *dims) -> AP:
    """Free-dim view of an SBUF tile AP: keep partition dim, set free dims.

    dims are (step, count) pairs in element units relative to the tile row.
    """
    return AP(base.tensor, base.offset + off,
              [list(base.ap[0])] + [[s, c] for (s, c) in dims])


def _emit(nc, tc, x_ap, w_ap, y_ap):
    f32 = F32
    vec = nc.vector
    act = nc.scalar
    STAGE = int(os.environ.get("KERNEL_STAGE", "99"))

    # b = p*TPC + t so that the output DMA is one fully-contiguous transfer
    x_flat = x_ap.rearrange("b c h w -> b (c h w)")
    x_tiled = x_flat.rearrange("(p t) f -> p t f", p=P)
    y_flat = y_ap.rearrange("b h w -> b (h w)").rearrange("(p t) e -> p (t e)", p=P)

    with tc.tile_pool(name="xin", bufs=3) as xpool, \
         tc.tile_pool(name="wk", bufs=1) as wp:
        # ---- W broadcast to all partitions --------------------------------
        w_row = wp.tile([P, C], f32)
        w_rep = wp.tile([P, C], f32)
        cam = wp.tile([P, TPC * E], f32)     # camera matrices, compact (t, e)
        scr = wp.tile([P, C], f32)           # ttr elementwise-out scratch

        if STAGE == 0:
            vec.memset(cam[:], 0.0)
            for t in range(TPC):
                xt = xpool.tile([P, C * E], f32, tag="xt", name=f"xt{t}")
                nc.sync.dma_start(out=xt[:], in_=x_tiled[:, t, :])
                vec.tensor_tensor(v(cam, t * E, (1, E)), v(xt, 0, (1, E)),
                                  v(xt, E, (1, E)), ALU.add)
            nc.sync.dma_start(out=y_flat, in_=v(cam, 0, (1, TPC * E)))
            return

        nc.sync.dma_start(out=w_row[:1, :], in_=AP(w_ap.tensor, 0, [[1, 1], [1, C]]))
        if STAGE == 1:
            vec.tensor_copy(w_rep[:1, :], w_row[:1, :])
        elif STAGE == 16:
            # broadcast via SBUF->SBUF DMA with step-0 partition source
            nc.sync.dma_start(out=w_rep[:],
                              in_=AP(w_row.tensor, w_row.offset,
                                     [[0, P], [1, C]]))
        else:
            nc.gpsimd.partition_broadcast(w_rep[:], w_row[:1, :])

        # ---- contraction: cam[p, t*9+h] = sum_c W[c] * x[b, c, h] ---------
        # w_big[c*9+j] = W[c], so prod = x_tile * w_big elementwise and the
        # camera entries are strided sums over c.
        w_big = wp.tile([P, C * E], f32)
        vec.tensor_copy(v(w_big, 0, (E, C), (1, E)), v(w_rep, 0, (1, C), (0, E)))
        for t in range(TPC):
            xt = xpool.tile([P, C * E], f32, tag="xt", name=f"xt{t}")
            nc.sync.dma_start(out=xt[:], in_=x_tiled[:, t, :])
            if STAGE == 1 or STAGE == 15 or STAGE == 16:
                vec.tensor_tensor(v(cam, t * E, (1, E)), v(xt, 0, (1, E)),
                                  w_rep[:, :E] if STAGE >= 15 else v(xt, E, (1, E)),
                                  ALU.mult)
                continue
            for h in range(E):
                vec.scalar_tensor_tensor(
                    scr[:], v(xt, h, (E, C)), 1.0, w_rep[:],
                    ALU.bypass, ALU.mult,
                    accum_out=v(cam, t * E + h, (1, 1)))

        # ---- SO(3) projection ---------------------------------------------
        NE = TPC * E                         # 288

        def mat(tile, off=0):
            # [P, (TPC, 3, 3)] compact view with offset into each 9-block
            return v(tile, off, (E, TPC), (3, 3), (1, 3))

        def flat(tile):
            return v(tile, 0, (1, NE))

        def row0(tile):
            return v(tile, 0, (E, TPC), (1, 3))

        def diag(tile):
            return v(tile, 0, (E, TPC), (4, 3))

        def pl(tile):
            return v(tile, 0, (1, TPC))

        def bc9(tile):
            # [P, TPC] plane broadcast over the 9 entries of each matrix
            return v(tile, 0, (1, TPC), (0, E))

        def bc3(tile):
            return v(tile, 0, (1, TPC), (0, 3))

        _consts = {}

        def cb(val):
            # [P, 1] constant tile for activation bias operands
            if val not in _consts:
                ct = wp.tile([P, 1], f32, name=f"const{len(_consts)}")
                vec.memset(ct[:], float(val))
                _consts[val] = ct[:]
            return _consts[val]

        Ya = wp.tile([P, NE], f32)
        Yb = wp.tile([P, NE], f32)
        D = wp.tile([P, TPC * 36], f32)
        Cf = wp.tile([P, NE], f32)
        t1 = wp.tile([P, NE], f32)
        t2 = wp.tile([P, NE], f32)
        t3 = wp.tile([P, NE], f32)
        td = wp.tile([P, TPC * 3], f32)
        det = wp.tile([P, TPC], f32)
        det0 = wp.tile([P, TPC], f32)
        s1p = wp.tile([P, TPC], f32)
        s2p = wp.tile([P, TPC], f32)
        s3p = wp.tile([P, TPC], f32)
        s4p = wp.tile([P, TPC], f32)
        u1 = wp.tile([P, TPC], f32)
        u2 = wp.tile([P, TPC], f32)
        u3 = wp.tile([P, TPC], f32)
        u4 = wp.tile([P, TPC], f32)

        def dblock(off):
            # view of D selecting D[a_block, b_block] as (TPC, 3, 3)
            return v(D, off, (36, TPC), (6, 3), (1, 3))

        def build_D(Y):
            # D[m] = [[Y, Y], [Y, Y]] as a 6x6 (row-major, stride 6)
            src = v(Y, 0, (E, TPC), (3, 3), (1, 3))
            for off in (0, 3, 18, 21):
                act.copy(v(D, off, (36, TPC), (6, 3), (1, 3)), src)

        def cofactor(Y, out):
            # out[i,j] = D[i+1,j+1]D[i+2,j+2] - D[i+1,j+2]D[i+2,j+1]
            build_D(Y)
            vec.tensor_tensor(mat(t1), dblock(7), dblock(14), ALU.mult)
            vec.tensor_tensor(mat(t2), dblock(8), dblock(13), ALU.mult)
            vec.tensor_tensor(mat(out), mat(t1), mat(t2), ALU.subtract)

        def det_of(Y, Cof, out):
            vec.tensor_tensor(v(td, 0, (3, TPC), (1, 3)), row0(Y), row0(Cof),
                              ALU.mult)
            vec.tensor_reduce(pl(out), v(td, 0, (3, TPC), (1, 3)),
                              mybir.AxisListType.X, ALU.add)

        if STAGE <= 2 or STAGE in (15, 16):
            nc.sync.dma_start(out=y_flat, in_=flat(cam))
            return

        # Newton polar iteration
        Y = cam
        other = [Ya, Yb]
        for it in range(min(NEWTON_ITERS, 99 if STAGE > 3 else 1)):
            cofactor(Y, Cf)
            det_of(Y, Cf, det)
            if it == 0:
                vec.tensor_copy(pl(det0), pl(det))
            Yn = other[it % 2]
            if it < SCALED_ITERS:
                # mu = |det|^(-1/3) = exp(-ln(det^2 + eps)/6)
                vec.tensor_tensor(pl(s1p), pl(det), pl(det), ALU.mult)
                act.activation(pl(s1p), pl(s1p), ACT.Ln, bias=cb(1e-35))
                act.activation(pl(s1p), pl(s1p), ACT.Exp, scale=-1.0 / 6.0, bias=cb(0.0))
                # s = 0.5/(mu*det);  muh = 0.5*mu
                vec.tensor_tensor(pl(s2p), pl(s1p), pl(det), ALU.mult)
                vec.reciprocal(pl(s2p), pl(s2p))
                vec.tensor_scalar_mul(pl(s2p), pl(s2p), 0.5)
                vec.tensor_scalar_mul(pl(s1p), pl(s1p), 0.5)
                vec.tensor_tensor(flat(t1), flat(Y), bc9(s1p), ALU.mult)
                vec.tensor_tensor(flat(t2), flat(Cf), bc9(s2p), ALU.mult)
                vec.tensor_tensor(flat(Yn), flat(t1), flat(t2), ALU.add)
            else:
                vec.reciprocal(pl(s2p), pl(det))
                vec.tensor_scalar_mul(pl(s2p), pl(s2p), 0.5)
                vec.tensor_scalar_mul(flat(t1), flat(Y), 0.5)
                vec.tensor_tensor(flat(t2), flat(Cf), bc9(s2p), ALU.mult)
                vec.tensor_tensor(flat(Yn), flat(t1), flat(t2), ALU.add)
            Y = Yn
        orth = Y

        if STAGE <= 4:
            nc.sync.dma_start(out=y_flat, in_=flat(orth))
            return

        # ---- reflection correction ---------------------------------------
        # P = orth^T @ cam  (into t3)
        Pm = t3
        for k in range(3):
            a = v(orth, 3 * k, (E, TPC), (1, 3), (0, 3))
            b = v(cam, 3 * k, (E, TPC), (0, 3), (1, 3))
            if k == 0:
                vec.tensor_tensor(mat(Pm), a, b, ALU.mult)
            else:
                vec.tensor_tensor(mat(t1), a, b, ALU.mult)
                vec.tensor_tensor(mat(Pm), mat(Pm), mat(t1), ALU.add)

        cofactor(Pm, Cf)                      # CP in Cf (uses t1, t2)
        c2 = s1p
        c1 = s2p
        c0 = s3p
        vec.tensor_reduce(pl(c2), diag(Pm), mybir.AxisListType.X, ALU.add)
        vec.tensor_reduce(pl(c1), diag(Cf), mybir.AxisListType.X, ALU.add)
        det_of(Pm, Cf, c0)

        q = det                               # reuse (det0 still holds sign info)
        p26 = wp.tile([P, TPC], f32)
        pp = wp.tile([P, TPC], f32)
        r = wp.tile([P, TPC], f32)
        s3 = s4p
        vec.tensor_scalar_mul(pl(q), pl(c2), 1.0 / 3.0)
        # p2/6 = ((2/3)c2^2 - 2 c1)/6 = c2^2/9 - c1/3
        vec.tensor_scalar_mul(pl(r), pl(c1), -1.0 / 3.0)
        vec.tensor_tensor(pl(p26), pl(c2), pl(c2), ALU.mult)
        vec.tensor_scalar_mul(pl(p26), pl(p26), 1.0 / 9.0)
        vec.tensor_tensor(pl(p26), pl(p26), pl(r), ALU.add)
        vec.tensor_scalar(pl(p26), pl(p26), 0.0, None, ALU.max)
        act.activation(pl(pp), pl(p26), ACT.Sqrt, bias=cb(1e-30))
        # detB = ((c2 - q)q - c1)q + c0 ; (c2 - q) = (2/3) c2
        vec.tensor_scalar_mul(pl(r), pl(c2), 2.0 / 3.0)
        vec.tensor_tensor(pl(r), pl(r), pl(q), ALU.mult)
        vec.tensor_tensor(pl(r), pl(r), pl(c1), ALU.subtract)
        vec.tensor_tensor(pl(r), pl(r), pl(q), ALU.mult)
        vec.tensor_tensor(pl(r), pl(r), pl(c0), ALU.add)     # r := detB
        # r = detB / (2 p^3 + eps)
        p3 = p26
        vec.tensor_tensor(pl(p3), pl(p26), pl(pp), ALU.mult)
        vec.tensor_scalar(pl(p3), pl(p3), 2.0, 1e-30, ALU.mult, ALU.add)
        vec.reciprocal(pl(p3), pl(p3))
        vec.tensor_tensor(pl(r), pl(r), pl(p3), ALU.mult)
        vec.tensor_scalar(pl(r), pl(r), -1.0, 1.0, ALU.max, ALU.min)
        # acos(r) via range-reduced atan (HW atan domain is [-pi/2, pi/2]):
        #   u = sqrt(1-r^2); phi = atan(min(|r|,u)/max(|r|,u)) in [0, pi/4]
        #   acos(r) = A + B*phi, A = pi/2*(1 - g + 2 s g), B = (1-2s)(2g-1)
        #   g = (|r| > u), s = (r < 0)
        vec.tensor_tensor(pl(u1), pl(r), pl(r), ALU.mult)
        act.activation(pl(u1), pl(u1), ACT.Sqrt, scale=-1.0, bias=cb(1.0 + 1e-12))
        vec.tensor_scalar_mul(pl(u2), pl(r), -1.0)
        vec.tensor_tensor(pl(u2), pl(u2), pl(r), ALU.max)         # |r|
        vec.tensor_tensor(pl(u3), pl(u2), pl(u1), ALU.min)
        vec.tensor_tensor(pl(u4), pl(u2), pl(u1), ALU.max)
        vec.reciprocal(pl(u4), pl(u4))
        vec.tensor_tensor(pl(u3), pl(u3), pl(u4), ALU.mult)
        act.activation(pl(u3), pl(u3), ACT.Arctan, bias=cb(0.0))
        zb = v(cb(0.0), 0, (0, TPC))
        vec.tensor_tensor(pl(u4), pl(u2), pl(u1), ALU.is_gt)      # g
        vec.tensor_tensor(pl(u2), pl(r), zb, ALU.is_lt)           # s
        vec.tensor_tensor(pl(u1), pl(u2), pl(u4), ALU.mult)       # s*g
        vec.tensor_scalar(pl(u1), pl(u1), np.pi, None, ALU.mult)
        vec.tensor_scalar(pl(r), pl(u4), -np.pi / 2.0, np.pi / 2.0,
                          ALU.mult, ALU.add)
        vec.tensor_tensor(pl(u1), pl(u1), pl(r), ALU.add)         # A
        vec.tensor_scalar(pl(u2), pl(u2), -2.0, 1.0, ALU.mult, ALU.add)
        vec.tensor_scalar(pl(u4), pl(u4), 2.0, -1.0, ALU.mult, ALU.add)
        vec.tensor_tensor(pl(u2), pl(u2), pl(u4), ALU.mult)       # B
        vec.tensor_tensor(pl(u3), pl(u3), pl(u2), ALU.mult)       # B*phi
        vec.tensor_tensor(pl(u1), pl(u1), pl(u3), ALU.add)        # acos(r)
        # s3 = q - 2 p sin(acos/3 + pi/6)   (== q + 2p cos(acos/3 + 2pi/3))
        act.activation(pl(u1), pl(u1), ACT.Sin, scale=1.0 / 3.0, bias=cb(np.pi / 6.0))
        vec.tensor_tensor(pl(u1), pl(pp), pl(u1), ALU.mult)
        vec.scalar_tensor_tensor(pl(s3), pl(u1), -2.0, pl(q), ALU.mult, ALU.add)

        # Newton polish on p(l) = -l^3 + c2 l^2 - c1 l + c0
        plv = pp
        dpl = r
        for _ in range(POLISH_ITERS):
            vec.tensor_tensor(pl(plv), pl(c2), pl(s3), ALU.subtract)
            vec.tensor_tensor(pl(plv), pl(plv), pl(s3), ALU.mult)
            vec.tensor_tensor(pl(plv), pl(plv), pl(c1), ALU.subtract)
            vec.tensor_tensor(pl(plv), pl(plv), pl(s3), ALU.mult)
            vec.tensor_tensor(pl(plv), pl(plv), pl(c0), ALU.add)
            vec.tensor_scalar(pl(dpl), pl(s3), -3.0, None, ALU.mult)
            vec.scalar_tensor_tensor(pl(dpl), pl(c2), 2.0, pl(dpl),
                                     ALU.mult, ALU.add)
            vec.tensor_tensor(pl(dpl), pl(dpl), pl(s3), ALU.mult)
            vec.tensor_tensor(pl(dpl), pl(dpl), pl(c1), ALU.subtract)
            vec.tensor_scalar(pl(dpl), pl(dpl), -1e-20, None, ALU.add)
            vec.reciprocal(pl(dpl), pl(dpl))
            vec.tensor_tensor(pl(plv), pl(plv), pl(dpl), ALU.mult)
            vec.tensor_tensor(pl(s3), pl(s3), pl(plv), ALU.subtract)

        # Nadj = CP + s3*P + (s3^2 - s3*c2) I
        w1 = q
        vec.tensor_tensor(pl(w1), pl(s3), pl(c2), ALU.mult)
        vec.tensor_tensor(pl(plv), pl(s3), pl(s3), ALU.mult)
        vec.tensor_tensor(pl(w1), pl(plv), pl(w1), ALU.subtract)
        vec.tensor_tensor(flat(t1), flat(Pm), bc9(s3), ALU.mult)
        vec.tensor_tensor(flat(Cf), flat(Cf), flat(t1), ALU.add)
        vec.tensor_tensor(diag(Cf), diag(Cf), bc3(w1), ALU.add)
        # proj = Nadj / (tr + eps)
        vec.tensor_reduce(pl(plv), diag(Cf), mybir.AxisListType.X, ALU.add)
        vec.tensor_scalar(pl(plv), pl(plv), 1e-30, None, ALU.add)
        vec.reciprocal(pl(plv), pl(plv))
        vec.tensor_tensor(flat(Cf), flat(Cf), bc9(plv), ALU.mult)
        # corr = orth @ proj
        corr = t3                              # Pm no longer needed
        for k in range(3):
            a = v(orth, k, (E, TPC), (3, 3), (0, 3))
            b = v(Cf, 3 * k, (E, TPC), (0, 3), (1, 3))
            if k == 0:
                vec.tensor_tensor(mat(corr), a, b, ALU.mult)
            else:
                vec.tensor_tensor(mat(t1), a, b, ALU.mult)
                vec.tensor_tensor(mat(corr), mat(corr), mat(t1), ALU.add)
        # f = 2*(det0 < 0);  R = orth - clamp(f*corr)
        vec.tensor_tensor(pl(plv), pl(det0), v(cb(0.0), 0, (0, TPC)), ALU.is_lt)
        vec.tensor_scalar_mul(pl(plv), pl(plv), 2.0)
        vec.tensor_tensor(flat(corr), flat(corr), bc9(plv), ALU.mult)
        vec.tensor_scalar(flat(corr), flat(corr), -2.0, 2.0, ALU.max, ALU.min)
        vec.tensor_tensor(flat(t1), flat(orth), flat(corr), ALU.subtract)

        nc.sync.dma_start(out=y_flat, in_=flat(t1))


def build(b_local=B_LOCAL):
    global TPC
    TPC = b_local // P
    nc = bacc.Bacc("TRN2", target_bir_lowering=False, debug=False)
    x = nc.dram_tensor("x", [b_local, C, 3, 3], F32, kind="ExternalInput")
    w = nc.dram_tensor("W", [C], F32, kind="ExternalInput")
    y = nc.dram_tensor("y", [b_local, 3, 3], F32, kind="ExternalOutput")
    with TileContext(nc) as tc:
        _emit(nc, tc, x.ap(), w.ap(), y.ap())
    nc.compile()
    return nc


_NC_CACHE = {}


def kernel(x: np.ndarray, W: np.ndarray) -> np.ndarray:
    assert x.shape == (B_FULL, C, 3, 3) and W.shape == (C,)
    if "nc" not in _NC_CACHE:
        _NC_CACHE["nc"] = build()
    nc = _NC_CACHE["nc"]
    xs = np.ascontiguousarray(x.reshape(N_CORES, B_LOCAL, C, 3, 3))
    in_maps = [{"x": xs[i], "W": W} for i in range(N_CORES)]
    res = bass_utils.run_bass_kernel_spmd(nc, in_maps, core_ids=list(range(N_CORES)))
    return np.concatenate([r["y"] for r in res.results], axis=0)


if __name__ == "__main__":
    rng = np.random.default_rng(0)
    x = rng.standard_normal((B_FULL, C, 3, 3), dtype=np.float32)
    W = (rng.standard_normal(C, dtype=np.float32) / np.sqrt(C)).astype(np.float32)
    out = kernel(x=x, W=W)
    print(out.shape, out.dtype)
